# revision 24
# baseline (speedup 1.0000x reference)
"""Causal GQA attention on 8 TRN2 NeuronCores.

Problem: q [4096, 4096] = [bs*seq, 32 heads * 128], k/v [4096, 1024] =
[bs*seq, 8 kv heads * 128], causal softmax(q k^T / sqrt(128)) v with GQA
(4 query heads per kv head). f32 in/out.

Sharding: 8 cores = 2 batches x 4 head-groups. Each core owns one batch
and 8 query heads / 2 kv heads -- fully local, no collectives. Q and K are
handed to each core pre-permuted to [head_dim, head, seq]; V is pre-packed
host-side as bf16 [key%128, block, kv, d|1] with the softmax-denominator
ones column baked in.

Per-core algorithm (Python-unrolled, Tile-scheduled):
  - QK: S^T[k, q] = K_j^T Q in bank-aligned <=512 chunks (contraction d on
    partitions). No mask add: exp runs on raw scores (bounded ~6 sigma) and
    GPSIMD affine_select zeroes the above-diagonal part of the diagonal
    P^T subtile after the fact.
  - exp is split across TWO engines to break the ACT throughput wall:
    ScalarE runs the Exp LUT; VectorE runs a 2-instruction custom-DVE
    pipeline p = horner4(s) ~ e^(s*scale/8) (fp16 scratch), then p^8 via
    3 squarings. Units are greedily load-balanced between the engines.
  - PV: P^T-stationary chains accumulate out[q, 0:128] plus the ones
    column denominator, THREE query subtiles per PSUM bank, so reciprocal
    and the normalize multiply are batched 3-wide (broadcast AP).

Walrus sync-wait limits (1 slot on DMA descriptors and LDWEIGHTS): all
loads land upfront in fresh buffers (zero waits), tiny PE warmup matmuls
absorb the DMA semaphores into PE's vector clock.
"""

import numpy as np

P = 128          # partitions / head_dim / key block
SEQ = 2048       # per-core sequence length
H = 8            # query heads per core
KV = 2           # kv heads per core
D = 128          # head dim
NB = SEQ // P    # 16 seq blocks
G = 2            # query groups per head
GW = SEQ // G    # 1024 query-group width
GB = GW // P     # 8 query subtiles per group
G3 = 3           # PV chains batched per PSUM bank for normalize
SCALE = float(D) ** -0.5

# exp(s*SCALE) = p(s)^8, p = 1 + s(A1 + s(A2 + s(A3 + s*A4))) minimax-fit
# to e^(s*SCALE/8) on |s| <= 104 (9.2 sigma of the N(0,128) logits);
# poly rel err 1.25e-3, x8 after the squarings => ~1e-2 at the extreme
# tails, ~1e-4 in the bulk. p^8 >= 0 always (even power), no overflow.
A1 = 0.011024713494762693
A2 = 6.152919028082445e-05
A3 = 2.4040177113194457e-07
A4 = 5.882085432516117e-10

_NC = None
_DVE_OPS = None


def _register_dve_ops():
    """Register the two custom DVE ops (idempotent per process)."""
    global _DVE_OPS
    if _DVE_OPS is not None:
        return _DVE_OPS
    import concourse.dve_ops as dve_ops
    from concourse.dve_spec import Spec, Src0, Src1, One, C0, C1, C2, sq, lower
    from concourse.dve_uop import DveOpSpec

    def _mk(name, body, reference, rd1, perf_en=False):
        if name in dve_ops._SUB_OPCODE_FOR_NAME:
            return next(o for o in dve_ops.OPS if o.name == name)
        opcode = max(dve_ops._SUB_OPCODE_FOR_NAME.values()) + 1
        spec = Spec(body=body, reference=reference)
        sha = DveOpSpec(name=name, opcode=opcode,
                        uops=lower(spec, ver="v3"), rd1_en=rd1).sha("v3")
        op = dve_ops.DveOp(name, spec, subdim=False, uops_sha={"v3": sha},
                           perf_en={"v3": True} if perf_en else {})
        dve_ops._SUB_OPCODE_FOR_NAME[name] = opcode
        dve_ops.OPS.append(op)
        dve_ops.CUSTOM_DVE_SPECS[name] = op.spec
        return op

    t = Src0
    exp8p = _mk(
        "EXP8P_ATTN",
        One + t * (C0 + t * (C1 + t * (C2 + t * Src1))),
        lambda in0, in1, s0, s1, imm2:
            1.0 + in0 * (s0 + in0 * (s1 + in0 * (imm2 + in0 * in1))),
        True,
    )
    import os as _os
    if _os.environ.get("KATTN_TOPS", "0") == "1":
        _mk("T_SRC1_ATTN", t * Src1,
            lambda in0, in1, s0, s1, imm2: in0 * in1, True)
        _mk("T_ONE_ATTN", One + t,
            lambda in0, in1, s0, s1, imm2: 1.0 + in0, False)
        _mk("T_DEEP_ATTN", sq(sq(sq(sq(sq(sq(sq(sq(t)))))))),
            lambda in0, in1, s0, s1, imm2: in0 ** 256, False)
        _mk("T_HORN_ATTN", C0 + t * (C1 + t * (C2 + t * (C0 + t * C1))),
            lambda in0, in1, s0, s1, imm2:
                s0 + in0 * (s1 + in0 * (imm2 + in0 * (s0 + in0 * s1))), False)
        _mk("T_ONE3_ATTN", One + (t * C0) * C1,
            lambda in0, in1, s0, s1, imm2: 1.0 + (in0 * s0) * s1, False)
        _mk("T_SRC13_ATTN", (t * C0) * Src1 + C1,
            lambda in0, in1, s0, s1, imm2: (in0 * s0) * in1 + s1, True)
    sq8 = _mk(
        "SQ8_ATTN",
        sq(sq(sq(t))),
        lambda in0, in1, s0, s1, imm2:
            ((in0 * in0) * (in0 * in0)) * ((in0 * in0) * (in0 * in0)),
        False,
    )
    _DVE_OPS = (exp8p, sq8)
    return _DVE_OPS


def _build_nc():
    import concourse.bass as bass
    import concourse.bacc as bacc
    import concourse.mybir as mybir
    import concourse.tile as tile
    from contextlib import ExitStack

    exp8p, sq8 = _register_dve_ops()

    f32 = mybir.dt.float32
    f16 = mybir.dt.float16
    bf16 = mybir.dt.bfloat16
    Exp = mybir.ActivationFunctionType.Exp

    nc = bacc.Bacc()
    qT_ext = nc.declare_dram_parameter("qT", [P, H, SEQ], bf16, isOutput=False)
    kT_ext = nc.declare_dram_parameter("kT", [P, KV, SEQ], bf16, isOutput=False)
    v_ext = nc.declare_dram_parameter("vones", [P, NB, KV, D + 1], bf16,
                                      isOutput=False)
    o_ext = nc.declare_dram_parameter("out", [SEQ, H * D], f32, isOutput=True)

    od = o_ext.rearrange("(i p) c -> p i c", p=P)

    def qk_chunks(a):
        """Bank-aligned <=512-wide chunks covering [a, GW); pieces narrower
        than 256 are widened leftward (within their bank) so matmuls stay
        near the 1 cycle/row rate. Widened cols hold garbage never read."""
        cs = []
        c = a
        while c < GW:
            nxt = min(GW, (c // 512 + 1) * 512)
            c0 = c
            if nxt - c0 < 256:
                c0 = max(nxt - 256, (nxt - 1) // 512 * 512)
            cs.append((c0, nxt))
            c = nxt
        return cs

    # unit list + greedy exp-engine balance (A = ScalarE LUT, V = DVE poly)
    import os
    dve_max = int(os.environ.get("KATTN_DVE", "999"))
    units = [(h, g, j) for h in range(H) for g in range(G)
             for j in range(GB * (g + 1))]
    assign = []
    act_load, dve_load = 0.0, 2000.0
    n_dve = 0
    for (h, g, j) in units:
        w = GW - max(0, j - GB * g) * P
        c_act = (w + 352) / 1.2
        c_dve = (2 * w + 178) / 0.96
        if j >= GB * g:
            dve_load += 220.0   # normalize/recip share of this diag's chain
        if n_dve >= dve_max or \
                max(act_load + c_act, dve_load) <= max(act_load, dve_load + c_dve):
            assign.append("A")
            act_load += c_act
        else:
            assign.append("V")
            dve_load += c_dve
            n_dve += 1
    # keep the tail units on ScalarE (shorter latency than the 2-pass DVE
    # pipeline; these gate the kernel's drain)
    for u in range(len(units) - 6, len(units)):
        assign[u] = "A"

    with ExitStack() as ctx:
        tc = ctx.enter_context(tile.TileContext(nc))
        singles = ctx.enter_context(tc.tile_pool(name="singles", bufs=1))
        pt_pool = ctx.enter_context(tc.tile_pool(name="pt", bufs=26))
        sc_pool = ctx.enter_context(tc.tile_pool(name="sc", bufs=3))
        ob_pool = ctx.enter_context(tc.tile_pool(name="ob", bufs=2))
        r_pool = ctx.enter_context(tc.tile_pool(name="r", bufs=6))
        ps_st = ctx.enter_context(tc.tile_pool(name="ps_st", bufs=3, space="PSUM"))
        ps_pv = ctx.enter_context(tc.tile_pool(name="ps_pv", bufs=2, space="PSUM"))

        # ---- upfront loads, spread across 4 issuing engines so descriptor
        # generation (~750ns each) doesn't serialize the first pieces ----
        kt = singles.tile([P, KV, SEQ], bf16)      # [d, kv, key]
        qt = singles.tile([P, H, SEQ], bf16)       # [d, head, query]
        vones = singles.tile([P, NB, KV, D + 1], bf16)  # [k, block, kv, d|1]
        # first-needed pieces first, tiny so unit 0 can start ASAP
        nc.sync.dma_start(out=kt[:, 0, 0:P], in_=kT_ext.ap()[:, 0, 0:P])
        nc.scalar.dma_start(out=qt[:, 0, 0:512], in_=qT_ext.ap()[:, 0, 0:512])
        nc.sync.dma_start(out=qt[:, 0, 512:GW], in_=qT_ext.ap()[:, 0, 512:GW])
        nc.gpsimd.dma_start(out=vones[:, 0:GB, 0, :],
                            in_=v_ext.ap()[:, 0:GB, 0, :])
        nc.sync.dma_start(out=kt[:, 0, P:GW], in_=kT_ext.ap()[:, 0, P:GW])
        nc.sync.dma_start(out=qt[:, 0, GW:], in_=qT_ext.ap()[:, 0, GW:])
        nc.gpsimd.dma_start(out=kt[:, 0, GW:], in_=kT_ext.ap()[:, 0, GW:])
        nc.gpsimd.dma_start(out=vones[:, GB:, 0, :],
                            in_=v_ext.ap()[:, GB:, 0, :])
        nc.gpsimd.dma_start(out=kt[:, 1:2, :], in_=kT_ext.ap()[:, 1:2, :])
        qeng = [nc.sync, nc.gpsimd]
        for i in range(1, H):
            qeng[(i - 1) % 2].dma_start(out=qt[:, i:i + 1, :],
                                        in_=qT_ext.ap()[:, i:i + 1, :])
        nc.gpsimd.dma_start(out=vones[:, :, 1, :], in_=v_ext.ap()[:, :, 1, :])

        # constant plane for the DVE poly's 4th coefficient ([P,1]-broadcast
        # Src1 hangs the DVE on HW; a full-width row works)
        a4full = singles.tile([P, GW], f32)
        nc.vector.memset(a4full, A4)
        a4col = a4full[:, 0:1]
        import os as _os
        _probe = _os.environ.get("KATTN_PROBE", "0")
        if _probe == "1":
            probe = singles.tile([P, 8], f32)
            nc.vector.memset(probe, 2.0)
            nc.vector.reciprocal_approx_fast(out=probe, in_=probe)
        elif _probe == "2":
            probe = singles.tile([P, 8], f32)
            nc.vector.memset(probe, 2.0)
            nc.vector._custom_dve(sq8, out=probe, in0=probe)
        elif _probe == "3":
            probe = singles.tile([P, 8], f32)
            nc.vector.memset(probe, 2.0)
            nc.vector._custom_dve(exp8p, out=probe, in0=probe,
                                  in1=a4col, s0=A1, s1=A2, imm2=A3)
        elif _probe in ("4", "5", "6", "7", "8", "9"):
            import concourse.dve_ops as _dv
            _name = {"4": "T_SRC1_ATTN", "5": "T_ONE_ATTN",
                     "6": "T_DEEP_ATTN", "7": "T_HORN_ATTN",
                     "8": "T_ONE3_ATTN", "9": "T_SRC13_ATTN"}[_probe]
            _op = next(o for o in _dv.OPS if o.name == _name)
            probe = singles.tile([P, 8], f32)
            nc.vector.memset(probe, 1.01)
            if _probe in ("4", "9"):
                nc.vector._custom_dve(_op, out=probe, in0=probe, in1=a4col,
                                      s0=0.1, s1=0.2)
            else:
                nc.vector._custom_dve(_op, out=probe, in0=probe,
                                      s0=0.1, s1=0.2, imm2=0.3)
        elif _probe == "10":
            import concourse.dve_ops as _dv
            _op = next(o for o in _dv.OPS if o.name == "T_SRC13_ATTN")
            probe = singles.tile([P, 8], f32)
            probe2 = singles.tile([P, 8], f32)
            nc.vector.memset(probe, 1.01)
            nc.vector.memset(probe2, 3.0)
            nc.vector._custom_dve(_op, out=probe, in0=probe, in1=probe2,
                                  s0=0.1, s1=0.2)

        # ---- PE warmups: absorb every load's semaphore into PE's clock so
        # real matmuls never carry a second (DMA) wait. Outputs unread.
        def warm(ap):
            n = ap.shape[-1]
            wm = ps_pv.tile([2, 2], f32, tag="pvacc", name="wm")
            nc.tensor.matmul(wm[:n, :n], lhsT=ap, rhs=ap, start=True, stop=True)

        warm(kt[:, 0, 0:1])
        warm(qt[:, 0, 0:1])
        warm(qt[:, 0, 512:513])
        warm(kt[:, 0, P:P + 1])
        warm(vones[:, 0, 0, 0:1])
        # ACT table warm (Exp set loads once, off the critical path)
        actwarm = sc_pool.tile([P, P], bf16, tag="actwarm", name="actwarm")
        nc.scalar.activation(out=actwarm, in_=nc.const_aps.tensor(0.0, (P, P)),
                             func=Exp, scale=SCALE)

        # ---- one global software pipeline over (head, group, key-block) ----
        LA = 2  # matches ps_st bufs=3: slots u..u+2 live
        sts = {}
        pts = {}
        o_sbs = {}

        def emit_qk(u):
            h, g, j = units[u]
            kvh = h // (H // KV)
            if g == 0 and j == 0:  # first touch of this head's q slice
                if h > 0:
                    warm(qt[:, h, 0:1])
                if h == H // KV:
                    warm(kt[:, 1, 0:1])
                    warm(vones[:, 0, 1, 0:1])
            if h == 0 and g == 1 and j == 0:  # second halves of kt0/qt0/v-kv0
                warm(qt[:, 0, GW:GW + 1])
                warm(kt[:, 0, GW:GW + 1])
                warm(vones[:, GB, 0, 0:1])
            s0 = max(0, j - GB * g)
            st = ps_st.tile([P, GW], f32, name="st")
            for (c0, c1) in qk_chunks(s0 * P):
                nc.tensor.matmul(
                    st[:, c0:c1],
                    lhsT=kt[:, kvh, j * P:(j + 1) * P],
                    rhs=qt[:, h, g * GW + c0:g * GW + c1],
                    start=True,
                    stop=True,
                )
            sts[u] = st

        CD = int(os.environ.get("KATTN_CD", "2"))  # chain delay in units
        pending = {}  # emit-at-unit -> list of (h, g, s)
        groups = {}   # h -> dict(acc, i0, n)

        def emit_chain(h, g, s):
            kvh = h // (H // KV)
            i_glob = g * GB + s
            gr = groups.get(h)
            if gr is None or gr["n"] == G3:
                gr = {"acc": ps_pv.tile([P, G3, D + 1], f32, name="pvacc"),
                      "i0": i_glob, "n": 0}
                groups[h] = gr
            ci = gr["n"]
            acc = gr["acc"]
            for j2 in range(GB * g + s + 1):
                nc.tensor.matmul(
                    acc[:, ci, :],
                    lhsT=pts[h][g][j2][:, s * P:(s + 1) * P],
                    rhs=vones[:, j2, kvh, :],
                    start=(j2 == 0),
                    stop=(j2 == GB * g + s),
                )
            gr["n"] += 1
            if gr["n"] == G3 or i_glob == NB - 1:
                n, i0 = gr["n"], gr["i0"]
                r = r_pool.tile([P, G3, 1], f32, name="r")
                nc.vector.reciprocal(r[:, 0:n, :], acc[:, 0:n, D:D + 1])
                nc.vector.tensor_mul(
                    o_sbs[h][:, i0:i0 + n, :],
                    acc[:, 0:n, 0:D],
                    r[:, 0:n, :].broadcast_to([P, n, D]),
                )
                nc.sync.dma_start(
                    out=od[:, i0:i0 + n, h * D:(h + 1) * D],
                    in_=o_sbs[h][:, i0:i0 + n, :],
                )
                if i_glob == NB - 1:
                    groups.pop(h)

        for u in range(min(LA, len(units))):
            emit_qk(u)
        for u in range(len(units)):
            if u + LA < len(units):
                emit_qk(u + LA)
            h, g, j = units[u]
            if g == 0 and j == 0:
                o_sbs[h] = ob_pool.tile([P, NB, D], f32, name="o_sb")
                pts[h] = {}
            s0 = max(0, j - GB * g)
            w = GW - s0 * P
            pt = pt_pool.tile([P, GW], bf16, name="pt")
            st = sts.pop(u)
            if assign[u] == "A":
                nc.scalar.activation(
                    out=pt[:, s0 * P:], in_=st[:, s0 * P:],
                    func=Exp, scale=SCALE
                )
            else:
                scr = sc_pool.tile([P, GW], f16, name="scr")
                nc.vector._custom_dve(
                    exp8p, out=scr[:, s0 * P:], in0=st[:, s0 * P:],
                    in1=a4full[:, s0 * P:], s0=A1, s1=A2, imm2=A3,
                )
                nc.vector._custom_dve(
                    sq8, out=pt[:, s0 * P:], in0=scr[:, s0 * P:],
                )
            if j >= GB * g:
                # zero the above-diagonal triangle of the diag subtile
                nc.gpsimd.affine_select(
                    out=pt[:, s0 * P:(s0 + 1) * P],
                    in_=pt[:, s0 * P:(s0 + 1) * P],
                    compare_op=mybir.AluOpType.is_ge,
                    fill=0.0,
                    base=0,
                    pattern=[[1, P]],   # keep where q_local - k_local >= 0
                    channel_multiplier=-1,
                )
            pts[h].setdefault(g, []).append(pt)
            if j >= GB * g:
                pending.setdefault(u + CD, []).append((h, g, j - GB * g))
            for ch in pending.pop(u, []):
                emit_chain(*ch)
        for u in sorted(pending):
            for ch in pending[u]:
                emit_chain(*ch)

    nc.compile()
    return nc


def _get_nc():
    global _NC
    if _NC is None:
        _NC = _build_nc()
    return _NC


def _shard_inputs(q, k, v):
    import ml_dtypes
    bf = ml_dtypes.bfloat16
    in_maps = []
    ones = np.ones((P, NB, KV, 1), np.float32)
    for c in range(8):
        b, hg = divmod(c, 4)
        rs = slice(b * SEQ, (b + 1) * SEQ)
        qs = q[rs, hg * 1024:(hg + 1) * 1024]    # [seq, 8*128]
        ks = k[rs, hg * 256:(hg + 1) * 256]      # [seq, 2*128]
        vs = v[rs, hg * 256:(hg + 1) * 256]      # [seq, 2*128]
        # [key%128, block, kv, d] + ones column
        vp = vs.reshape(NB, P, KV, D).transpose(1, 0, 2, 3)
        vo = np.concatenate([vp, ones], axis=3)
        in_maps.append({
            "qT": np.ascontiguousarray(
                qs.reshape(SEQ, H, D).transpose(2, 1, 0)
            ).astype(bf),
            "kT": np.ascontiguousarray(
                ks.reshape(SEQ, KV, D).transpose(2, 1, 0)
            ).astype(bf),
            "vones": np.ascontiguousarray(vo).astype(bf),
        })
    return in_maps


def _run(q, k, v, **spmd_kwargs):
    from concourse.bass_utils import run_bass_kernel_spmd

    nc = _get_nc()
    bkr = run_bass_kernel_spmd(nc, _shard_inputs(q, k, v),
                               core_ids=list(range(8)), **spmd_kwargs)
    out = np.empty((2 * SEQ, 32 * D), np.float32)
    for c in range(8):
        b, hg = divmod(c, 4)
        out[b * SEQ:(b + 1) * SEQ, hg * 1024:(hg + 1) * 1024] = \
            bkr.results[c]["out"]
    return out, bkr


def kernel(q, k, v, bs=2, seq_len=2048, **_ignored):
    q = np.asarray(q, dtype=np.float32)
    k = np.asarray(k, dtype=np.float32)
    v = np.asarray(v, dtype=np.float32)
    assert int(bs) == 2 and int(seq_len) == SEQ
    assert q.shape == (4096, 4096) and k.shape == (4096, 1024)
    out, _ = _run(q, k, v)
    return out
